# revision 30
# baseline (speedup 1.0000x reference)
"""Lorenz-96 vector field kernel for Trainium2 (8 NeuronCores, SPMD data-parallel).

field[..., i] = p0[i]*(state[i+1] - state[i-2])*state[i-1] - p1[i]*state[i] + p2[i]
(circular along the last axis, dim=256)

Sharding: batch axis (262144 rows) split evenly across 8 cores; params replicated.

This version runs the whole pipeline in fp16 (inputs are cast on the host):
halving the dtype halves both the HBM traffic on-device (memory-bound regime)
and the host<->device transfer volume, while keeping the relative error around
1e-3 - far below the 2e-2 gate.

Per-core layout: each SBUF partition holds R=32 batch rows as one flat stream of
R*259 halved floats: every row is [halo2 | 256 cols | halo1] where the 3-wide
halo carries the circular wrap (s[254], s[255] on the left, s[0] on the right).
All shifted stencil operands are contiguous flat views of the stream, so every
elementwise op uses the plain 2D tensor_tensor/STT encodings. Halo lanes compute
garbage that is never stored - the output DMA reads only the 256 real columns.

Engine split (6 irreducible two-tensor elementwise ops, balanced by the
cost-model rates DVE fp16 2x-mode 0.52 ns/elem vs GPSIMD 1.98 ns/elem):
  GPSIMD: diff = s[+1]-s[-2], plus ~12% of ops 5,6
  DVE:    um1 = p0*s[-1]; z = diff*um1; v' = (-p1)*s; ~88% of
          f = z + v' and out = f + p2   (p1 negated on host so op5 is an add)
  ACT:    per-tile halo fills + 1/3 of the one-time param-stream build
          (the other 2 planes build on DVE so tile 0 isn't blocked)
The param streams (259-periodic) are built on-device from a tiny [128,3,260]
upload so the wire only carries ~1.6 MB of params instead of ~50 MB.
(Tried and rejected: scalar_tensor_tensor on GPSIMD - walrus rejects
TensorScalarPtr on Pool; SWDGE accumulate-ADD DMAs for ops 5/6 - the
per-DMA DGE+semaphore latency in the dependency chain cost more than the
engine time saved.)

Host runner: a cached jax.jit(shard_map(...)) around the bass custom call - no
per-call retracing, no per-call 256 MB zero upload (the output buffer is
donation-chained from the previous call; the first one is created on-device).
"""

import numpy as np

import concourse.bass as bass
import concourse.mybir as mybir
from concourse.tile import TileContext
from concourse.vector_clock import ScopedClock, VectorClock


class SplitDrainTileContext(TileContext):
    """The kernel-tail Drain aggregates one sem wait per outstanding proc
    (compute engines + every HWDGE queue used); walrus rejects instructions
    with more than a couple of encoded waits. Pre-observe each proc with its
    own single-wait SP nop so the real drain needs none."""

    def _drain_and_barrier(self, tick_clock, wait_clock):
        full = tick_clock.global_clock
        n = len(list(full))
        for p in range(n):
            if full[p] == 0:
                continue
            partial = VectorClock([full[q] if q == p else 0 for q in range(n)])
            nop = self.nc.sync.nop(nofuse=True)
            wait_clock.add_sem_waits(nop.ins, ScopedClock({None: partial}))
        self.nc.sync.drain()
        self.nc.all_engine_barrier()
        assert self.sems is not None
        popped = self.nc._tile_sem_poison_stack.pop()
        assert popped is self._sem_poison
        self.nc.clear_and_free_semaphores(list(self.sems.allocated().values()))
        self.nc.all_engine_barrier()


def _split_waits(nc, limit: int = 1):
    """Post-lowering pass: walrus caps encoded sem waits per instruction
    (TT allows 1, DMACopy ~2). Move excess waits onto same-engine NoOps
    inserted immediately before the instruction - sequencers issue in
    order, so waiting earlier on the same stream preserves ordering."""
    for bb in nc.m.functions[0].blocks:
        il = bb.instructions
        i = 0
        while i < len(il):
            ins = il[i]
            si = getattr(ins, "sync_info", None)
            if si is not None and len(si.on_wait) > limit:
                waits = list(si.on_wait)
                keep, excess = waits[-limit:], waits[:-limit]
                for j, w in enumerate(excess):
                    nop = mybir.InstNoOp(
                        name=f"{ins.name}-wsplit{j}", ins=[], outs=[]
                    )
                    nop.engine = ins.engine
                    nop.sync_info = mybir.SyncInfo(on_wait=[w], on_update=[])
                    il.insert(i, nop)
                    i += 1
                ins.sync_info = mybir.SyncInfo(on_wait=keep, on_update=si.on_update)
            i += 1


P = 128          # SBUF partitions
DIM = 256        # Lorenz-96 dimension (stencil axis, unsharded)
EXT = DIM + 3    # per-row stream width incl. halo
PBW = 260        # padded per-plane width of the uploaded param periods (even)
NCORES = 8
R = 16           # batch rows per partition per tile
F16 = mybir.dt.float16


DVE_FRAC = 0.876  # DVE share of the op5/op6 lanes (rest on GPSIMD)


def build_nc(rows: int, r: int = R, dve_frac: float = DVE_FRAC,
             split_waits: bool = True, bufs: int = 3):
    """Build the per-core Bass program. `rows` = batch rows per core."""
    assert rows % (P * r) == 0
    nt = rows // (P * r)
    W = r * EXT          # flat stream width per partition (real data)
    WP = W + 4           # allocated width (views may peek past W; keep even)
    G0, G1 = 2, W - 1    # compute range (shifts -2..+1 stay in bounds)
    # flat DVE/GPSIMD split point (4-byte aligned for the 16-bit 2x mode)
    SD = G0 + int(round(dve_frac * (G1 - G0) / 2)) * 2

    nc = bass.Bass()
    st = nc.declare_dram_parameter("state", [rows, DIM], F16, isOutput=False)
    pbs = nc.declare_dram_parameter("pbs", [P, 3, PBW], F16, isOutput=False)
    out = nc.declare_dram_parameter("out", [rows, DIM], F16, isOutput=True)

    st_t = st.rearrange("(n p r) d -> n p r d", p=P, r=r)
    out_t = out.rearrange("(n p r) d -> n p r d", p=P, r=r)

    with SplitDrainTileContext(nc) as tc:
        with (
            tc.tile_pool(name="pp", bufs=1) as ppool,
            tc.tile_pool(name="ext", bufs=bufs) as extpool,
            tc.tile_pool(name="mid", bufs=bufs) as midpool,
            tc.tile_pool(name="op", bufs=bufs) as opool,
        ):
            # --- one-time param-stream build -------------------------------
            # Upload one 259-period per plane (padded to 260), then double it
            # out to the full r-period stream on the (otherwise idle) ACT.
            pbsb = ppool.tile([P, 3 * PBW], F16)
            nc.sync.dma_start(out=pbsb[:], in_=pbs.rearrange("p a w -> p (a w)"))
            pbt = ppool.tile([P, 3 * WP], F16)

            def build_plane(k, eng_copy):
                base = k * WP
                eng_copy(
                    pbt[:, base : base + EXT],
                    pbsb[:, k * PBW : k * PBW + EXT],
                )
                ln = EXT
                while ln < W:
                    cp = min(ln, W - ln)
                    eng_copy(
                        pbt[:, base + ln : base + ln + cp],
                        pbt[:, base : base + cp],
                    )
                    ln += cp

            # planes 0,1 on DVE (needed first, DVE 4x fp16 copies are cheap);
            # plane 2 on ACT, emitted after tile 0's halo fill below so the
            # first tile's stencil ops aren't stuck behind the param build.
            build_plane(0, nc.vector.tensor_copy)
            build_plane(1, nc.vector.tensor_copy)
            P0 = pbt[:, 0 * WP + G0 : 0 * WP + G1]
            P1 = pbt[:, 1 * WP + G0 : 1 * WP + G1]  # holds -p1 (host-negated)
            P2 = pbt[:, 2 * WP + G0 : 2 * WP + G1]

            for i in range(nt):
                ext = extpool.tile([P, WP], F16, tag="ext")
                e3 = ext[:, 0:W].rearrange("p (r c) -> p r c", c=EXT)
                nc.sync.dma_start(out=e3[:, :, 2 : DIM + 2], in_=st_t[i])
                # halo fill on ACT: left 2 cols = state[254:256], right = state[0]
                nc.scalar.copy(e3[:, :, 0:2], e3[:, :, DIM : DIM + 2])
                nc.scalar.copy(e3[:, :, DIM + 2 : DIM + 3], e3[:, :, 2:3])
                if i == 0:
                    build_plane(2, nc.scalar.copy)

                A = ext[:, G0:G1]              # s[c]
                Am1 = ext[:, G0 - 1 : G1 - 1]  # s[c-1]
                Am2 = ext[:, G0 - 2 : G1 - 2]  # s[c-2]
                Ap1 = ext[:, G0 + 1 : G1 + 1]  # s[c+1]

                um1 = midpool.tile([P, WP], F16, tag="um1")
                diff = midpool.tile([P, WP], F16, tag="diff")
                ot = opool.tile([P, WP], F16, tag="o")

                # op1 (GPSIMD): diff[c] = s[c+1] - s[c-2]
                nc.gpsimd.tensor_sub(diff[:, G0:G1], Ap1, Am2)
                # op2 (DVE): um1[c] = p0[c] * s[c-1]
                nc.vector.tensor_mul(um1[:, G0:G1], Am1, P0)
                # op3 (DVE): z = diff * um1   (in-place into um1)
                nc.vector.tensor_mul(um1[:, G0:G1], diff[:, G0:G1], um1[:, G0:G1])
                # op4 (DVE): v'[c] = -p1[c] * s[c]  (reuse diff tile)
                nc.vector.tensor_mul(diff[:, G0:G1], A, P1)
                # op5 (split at SD): f = z + v'  (in-place into um1)
                nc.vector.tensor_add(um1[:, G0:SD], um1[:, G0:SD], diff[:, G0:SD])
                nc.gpsimd.tensor_add(um1[:, SD:G1], um1[:, SD:G1], diff[:, SD:G1])
                # op6 (split at SD): out = f + p2
                nc.vector.tensor_add(
                    ot[:, G0:SD], um1[:, G0:SD], pbt[:, 2 * WP + G0 : 2 * WP + SD]
                )
                nc.gpsimd.tensor_add(
                    ot[:, SD:G1], um1[:, SD:G1], pbt[:, 2 * WP + SD : 2 * WP + G1]
                )

                o3 = ot[:, 0:W].rearrange("p (r c) -> p r c", c=EXT)
                nc.sync.dma_start(out=out_t[i], in_=o3[:, :, 2 : DIM + 2])

    if split_waits:
        _split_waits(nc)
    return nc


def make_pbs(params: np.ndarray) -> np.ndarray:
    """Host-side param prep: one 259-period per plane, padded to 260,
    broadcast to 128 partitions. Tiny (~200 KB fp16 per core).
    p1 is negated so the on-device '- p1*s' term becomes a tensor add."""
    pp = np.asarray(params, dtype=np.float32).copy()
    pp[1] = -pp[1]
    row = np.zeros((3, PBW), np.float16)
    row[:, 2 : DIM + 2] = pp.astype(np.float16)
    return np.ascontiguousarray(np.broadcast_to(row[None], (P, 3, PBW)))


_rt_cache: dict = {}


def _get_runtime(rows: int):
    """Build-and-cache the jitted SPMD executable for a per-core row count."""
    if rows in _rt_cache:
        return _rt_cache[rows]

    import jax
    import jax.numpy as jnp
    from jax.sharding import Mesh, PartitionSpec, NamedSharding

    try:
        from jax.experimental.shard_map import shard_map
    except ImportError:  # newer jax
        from jax import shard_map  # type: ignore

    from concourse import bass2jax

    nc = build_nc(rows)
    bass2jax.install_neuronx_cc_hook()

    partition_name = (
        nc.partition_id_tensor.name if nc.partition_id_tensor else None
    )
    in_names: list[str] = []
    out_names: list[str] = []
    out_avals = []
    for alloc in nc.m.functions[0].allocations:
        if not isinstance(alloc, mybir.MemoryLocationSet):
            continue
        name = alloc.memorylocations[0].name
        if alloc.kind == "ExternalInput":
            if name != partition_name:
                in_names.append(name)
        elif alloc.kind == "ExternalOutput":
            out_names.append(name)
            shape = tuple(alloc.tensor_shape)
            dtype = mybir.dt.np(alloc.dtype)
            out_avals.append(jax.core.ShapedArray(shape, dtype))
    n_params = len(in_names)
    n_outs = len(out_names)
    all_names = list(in_names) + list(out_names)
    if partition_name is not None:
        all_names.append(partition_name)
    all_names = tuple(all_names)

    def _body(*args):
        operands = list(args)
        if partition_name is not None:
            operands.append(bass2jax.partition_id_tensor())
        outs = bass2jax._bass_exec_p.bind(
            *operands,
            out_avals=tuple(out_avals),
            in_names=all_names,
            out_names=tuple(out_names),
            lowering_input_output_aliases=(),
            sim_require_finite=True,
            sim_require_nnan=True,
            nc=nc,
        )
        return tuple(outs)

    devices = jax.devices()[:NCORES]
    mesh = Mesh(np.asarray(devices), ("core",))
    pspec = PartitionSpec("core")
    sharded = jax.jit(
        shard_map(
            _body,
            mesh=mesh,
            in_specs=(pspec,) * (n_params + n_outs),
            out_specs=(pspec,) * n_outs,
            check_rep=False,
        ),
        donate_argnums=tuple(range(n_params, n_params + n_outs)),
        keep_unused=True,
    )
    B = rows * NCORES
    make_donor = jax.jit(
        lambda: jnp.zeros((B, DIM), jnp.float16),
        out_shardings=NamedSharding(mesh, pspec),
    )
    rt = {
        "sharded": sharded,
        "donor": make_donor(),
        "in_names": in_names,
        "sharding": NamedSharding(mesh, pspec),
    }
    _rt_cache[rows] = rt
    return rt


def _cast_parallel(src: np.ndarray, dst: np.ndarray, nthreads: int = 8):
    """dst[:] = src (dtype-converting), parallelized over row slices."""
    from concurrent.futures import ThreadPoolExecutor

    n = src.shape[0]
    step = (n + nthreads - 1) // nthreads
    def work(i):
        np.copyto(dst[i : i + step], src[i : i + step], casting="unsafe")
    with ThreadPoolExecutor(nthreads) as ex:
        list(ex.map(work, range(0, n, step)))
    return dst


_stage16: dict = {}


def kernel(state: np.ndarray, params: np.ndarray, t: np.ndarray = None) -> np.ndarray:
    state = np.asarray(state)
    params = np.asarray(params, dtype=np.float32)
    B = state.shape[0]
    rows = B // NCORES
    rt = _get_runtime(rows)

    if state.dtype == np.float16:
        state16 = np.ascontiguousarray(state)
    else:
        if B not in _stage16:
            _stage16[B] = np.empty((B, DIM), np.float16)
        state16 = _cast_parallel(state, _stage16[B])
    pbs1 = make_pbs(params)
    pbs_all = np.ascontiguousarray(
        np.broadcast_to(pbs1[None], (NCORES, P, 3, PBW))
    ).reshape(NCORES * P, 3, PBW)

    (out16,) = rt["sharded"](state16, pbs_all, rt["donor"])
    rt["donor"] = out16  # donation chain: reuse this device buffer next call
    host16 = np.asarray(out16)
    out32 = np.empty((B, DIM), np.float32)
    return _cast_parallel(host16, out32)


# revision 33
# speedup vs baseline: 1.0708x; 1.0708x over previous
"""Lorenz-96 vector field kernel for Trainium2 (8 NeuronCores, SPMD data-parallel).

field[..., i] = p0[i]*(state[i+1] - state[i-2])*state[i-1] - p1[i]*state[i] + p2[i]
(circular along the last axis, dim=256)

Sharding: batch axis (262144 rows) split evenly across 8 cores; params replicated.

This version runs the whole pipeline in fp16 (inputs are cast on the host):
halving the dtype halves both the HBM traffic on-device (memory-bound regime)
and the host<->device transfer volume, while keeping the relative error around
1e-3 - far below the 2e-2 gate.

Per-core layout: each SBUF partition holds R=32 batch rows as one flat stream of
R*259 halved floats: every row is [halo2 | 256 cols | halo1] where the 3-wide
halo carries the circular wrap (s[254], s[255] on the left, s[0] on the right).
All shifted stencil operands are contiguous flat views of the stream, so every
elementwise op uses the plain 2D tensor_tensor/STT encodings. Halo lanes compute
garbage that is never stored - the output DMA reads only the 256 real columns.

Engine split (6 irreducible two-tensor elementwise ops, balanced by the
cost-model rates DVE fp16 2x-mode 0.52 ns/elem vs GPSIMD 1.98 ns/elem):
  GPSIMD: diff = s[+1]-s[-2], plus ~12% of ops 5,6
  DVE:    um1 = p0*s[-1]; z = diff*um1; v' = (-p1)*s; ~88% of
          f = z + v' and out = f + p2   (p1 negated on host so op5 is an add)
  ACT:    per-tile halo fills + 1/3 of the one-time param-stream build
          (the other 2 planes build on DVE so tile 0 isn't blocked)
The param streams (259-periodic) are built on-device from a tiny [128,3,260]
upload so the wire only carries ~1.6 MB of params instead of ~50 MB.
(Tried and rejected: scalar_tensor_tensor on GPSIMD - walrus rejects
TensorScalarPtr on Pool; SWDGE accumulate-ADD DMAs for ops 5/6 - the
per-DMA DGE+semaphore latency in the dependency chain cost more than the
engine time saved.)

Host runner: a cached jax.jit(shard_map(...)) around the bass custom call - no
per-call retracing, no per-call 256 MB zero upload (the output buffer is
donation-chained from the previous call; the first one is created on-device).
"""

import numpy as np

import concourse.bass as bass
import concourse.mybir as mybir
from concourse.tile import TileContext
from concourse.vector_clock import ScopedClock, VectorClock


class SplitDrainTileContext(TileContext):
    """The kernel-tail Drain aggregates one sem wait per outstanding proc
    (compute engines + every HWDGE queue used); walrus rejects instructions
    with more than a couple of encoded waits. Pre-observe each proc with its
    own single-wait SP nop so the real drain needs none."""

    def _drain_and_barrier(self, tick_clock, wait_clock):
        full = tick_clock.global_clock
        n = len(list(full))
        for p in range(n):
            if full[p] == 0:
                continue
            partial = VectorClock([full[q] if q == p else 0 for q in range(n)])
            nop = self.nc.sync.nop(nofuse=True)
            wait_clock.add_sem_waits(nop.ins, ScopedClock({None: partial}))
        self.nc.sync.drain()
        self.nc.all_engine_barrier()
        assert self.sems is not None
        popped = self.nc._tile_sem_poison_stack.pop()
        assert popped is self._sem_poison
        self.nc.clear_and_free_semaphores(list(self.sems.allocated().values()))
        self.nc.all_engine_barrier()


def _split_waits(nc, limit: int = 1):
    """Post-lowering pass: walrus caps encoded sem waits per instruction
    (TT allows 1, DMACopy ~2). Move excess waits onto same-engine NoOps
    inserted immediately before the instruction - sequencers issue in
    order, so waiting earlier on the same stream preserves ordering."""
    for bb in nc.m.functions[0].blocks:
        il = bb.instructions
        i = 0
        while i < len(il):
            ins = il[i]
            si = getattr(ins, "sync_info", None)
            if si is not None and len(si.on_wait) > limit:
                waits = list(si.on_wait)
                keep, excess = waits[-limit:], waits[:-limit]
                for j, w in enumerate(excess):
                    nop = mybir.InstNoOp(
                        name=f"{ins.name}-wsplit{j}", ins=[], outs=[]
                    )
                    nop.engine = ins.engine
                    nop.sync_info = mybir.SyncInfo(on_wait=[w], on_update=[])
                    il.insert(i, nop)
                    i += 1
                ins.sync_info = mybir.SyncInfo(on_wait=keep, on_update=si.on_update)
            i += 1


P = 128          # SBUF partitions
DIM = 256        # Lorenz-96 dimension (stencil axis, unsharded)
EXT = DIM + 3    # per-row stream width incl. halo
PBW = 260        # padded per-plane width of the uploaded param periods (even)
NCORES = 8
R = 16           # batch rows per partition per tile
F16 = mybir.dt.float16
ADD = mybir.AluOpType.add


DVE_FRAC = 0.876  # DVE share of the op5/op6 lanes (rest on GPSIMD)


def build_nc(rows: int, r: int = R, dve_frac: float = DVE_FRAC,
             split_waits: bool = True, bufs: int = 3, accum_tail: bool = False):
    """Build the per-core Bass program. `rows` = batch rows per core."""
    assert rows % (P * r) == 0
    nt = rows // (P * r)
    W = r * EXT          # flat stream width per partition (real data)
    WP = W + 4           # allocated width (views may peek past W; keep even)
    G0, G1 = 2, W - 1    # compute range (shifts -2..+1 stay in bounds)
    # flat DVE/GPSIMD split point (4-byte aligned for the 16-bit 2x mode)
    SD = G0 + int(round(dve_frac * (G1 - G0) / 2)) * 2

    nc = bass.Bass()
    st = nc.declare_dram_parameter("state", [rows, DIM], F16, isOutput=False)
    pbs = nc.declare_dram_parameter("pbs", [P, 3, PBW], F16, isOutput=False)
    out = nc.declare_dram_parameter("out", [rows, DIM], F16, isOutput=True)

    st_t = st.rearrange("(n p r) d -> n p r d", p=P, r=r)
    out_t = out.rearrange("(n p r) d -> n p r d", p=P, r=r)

    with SplitDrainTileContext(nc) as tc:
        with (
            tc.tile_pool(name="pp", bufs=1) as ppool,
            tc.tile_pool(name="ext", bufs=bufs) as extpool,
            tc.tile_pool(name="mid", bufs=bufs) as midpool,
            tc.tile_pool(name="op", bufs=bufs) as opool,
        ):
            # --- one-time param-stream build -------------------------------
            # Upload one 259-period per plane (padded to 260), then double it
            # out to the full r-period stream on the (otherwise idle) ACT.
            pbsb = ppool.tile([P, 3 * PBW], F16)
            nc.sync.dma_start(out=pbsb[:], in_=pbs.rearrange("p a w -> p (a w)"))
            pbt = ppool.tile([P, 3 * WP], F16)

            def build_plane(k, eng_copy):
                base = k * WP
                eng_copy(
                    pbt[:, base : base + EXT],
                    pbsb[:, k * PBW : k * PBW + EXT],
                )
                ln = EXT
                while ln < W:
                    cp = min(ln, W - ln)
                    eng_copy(
                        pbt[:, base + ln : base + ln + cp],
                        pbt[:, base : base + cp],
                    )
                    ln += cp

            # planes 0,1 on DVE (needed first, DVE 4x fp16 copies are cheap);
            # plane 2 on ACT, emitted after tile 0's halo fill below so the
            # first tile's stencil ops aren't stuck behind the param build.
            build_plane(0, nc.vector.tensor_copy)
            build_plane(1, nc.vector.tensor_copy)
            P0 = pbt[:, 0 * WP + G0 : 0 * WP + G1]
            P1 = pbt[:, 1 * WP + G0 : 1 * WP + G1]  # holds -p1 (host-negated)
            P2 = pbt[:, 2 * WP + G0 : 2 * WP + G1]

            for i in range(nt):
                ext = extpool.tile([P, WP], F16, tag="ext")
                e3 = ext[:, 0:W].rearrange("p (r c) -> p r c", c=EXT)
                nc.sync.dma_start(out=e3[:, :, 2 : DIM + 2], in_=st_t[i])
                # halo fill on ACT: left 2 cols = state[254:256], right = state[0]
                nc.scalar.copy(e3[:, :, 0:2], e3[:, :, DIM : DIM + 2])
                nc.scalar.copy(e3[:, :, DIM + 2 : DIM + 3], e3[:, :, 2:3])
                if i == 0:
                    build_plane(2, nc.scalar.copy)

                A = ext[:, G0:G1]              # s[c]
                Am1 = ext[:, G0 - 1 : G1 - 1]  # s[c-1]
                Am2 = ext[:, G0 - 2 : G1 - 2]  # s[c-2]
                Ap1 = ext[:, G0 + 1 : G1 + 1]  # s[c+1]

                um1 = midpool.tile([P, WP], F16, tag="um1")
                diff = midpool.tile([P, WP], F16, tag="diff")
                ot = opool.tile([P, WP], F16, tag="o")

                # op1 (GPSIMD): diff[c] = s[c+1] - s[c-2]
                nc.gpsimd.tensor_sub(diff[:, G0:G1], Ap1, Am2)
                # op2 (DVE): um1[c] = p0[c] * s[c-1]
                nc.vector.tensor_mul(um1[:, G0:G1], Am1, P0)
                # op3 (DVE): z = diff * um1   (in-place into um1)
                nc.vector.tensor_mul(um1[:, G0:G1], diff[:, G0:G1], um1[:, G0:G1])
                # op4 (DVE): v'[c] = -p1[c] * s[c]  (reuse diff tile)
                nc.vector.tensor_mul(diff[:, G0:G1], A, P1)
                if accum_tail:
                    # op6' (split at SD): t = z + p2
                    nc.vector.tensor_add(
                        ot[:, G0:SD], um1[:, G0:SD],
                        pbt[:, 2 * WP + G0 : 2 * WP + SD],
                    )
                    nc.gpsimd.tensor_add(
                        ot[:, SD:G1], um1[:, SD:G1],
                        pbt[:, 2 * WP + SD : 2 * WP + G1],
                    )
                    # op5' (SDMA CCE): out = t + v'. Only the out-DMA consumes
                    # this, so no engine ever stalls on the accum latency.
                    nc.gpsimd.dma_start(
                        out=ot[:, G0:G1], in_=diff[:, G0:G1], accum_op=ADD
                    )
                else:
                    # op5 (split at SD): f = z + v'  (in-place into um1)
                    nc.vector.tensor_add(
                        um1[:, G0:SD], um1[:, G0:SD], diff[:, G0:SD]
                    )
                    nc.gpsimd.tensor_add(
                        um1[:, SD:G1], um1[:, SD:G1], diff[:, SD:G1]
                    )
                    # op6 (split at SD): out = f + p2
                    nc.vector.tensor_add(
                        ot[:, G0:SD], um1[:, G0:SD],
                        pbt[:, 2 * WP + G0 : 2 * WP + SD],
                    )
                    nc.gpsimd.tensor_add(
                        ot[:, SD:G1], um1[:, SD:G1],
                        pbt[:, 2 * WP + SD : 2 * WP + G1],
                    )

                o3 = ot[:, 0:W].rearrange("p (r c) -> p r c", c=EXT)
                nc.sync.dma_start(out=out_t[i], in_=o3[:, :, 2 : DIM + 2])

    if split_waits:
        _split_waits(nc)
    return nc


def make_pbs(params: np.ndarray) -> np.ndarray:
    """Host-side param prep: one 259-period per plane, padded to 260,
    broadcast to 128 partitions. Tiny (~200 KB fp16 per core).
    p1 is negated so the on-device '- p1*s' term becomes a tensor add."""
    pp = np.asarray(params, dtype=np.float32).copy()
    pp[1] = -pp[1]
    row = np.zeros((3, PBW), np.float16)
    row[:, 2 : DIM + 2] = pp.astype(np.float16)
    return np.ascontiguousarray(np.broadcast_to(row[None], (P, 3, PBW)))


_rt_cache: dict = {}


def _get_runtime(rows: int):
    """Build-and-cache the jitted SPMD executable for a per-core row count."""
    if rows in _rt_cache:
        return _rt_cache[rows]

    import jax
    import jax.numpy as jnp
    from jax.sharding import Mesh, PartitionSpec, NamedSharding

    try:
        from jax.experimental.shard_map import shard_map
    except ImportError:  # newer jax
        from jax import shard_map  # type: ignore

    from concourse import bass2jax

    nc = build_nc(rows)
    bass2jax.install_neuronx_cc_hook()

    partition_name = (
        nc.partition_id_tensor.name if nc.partition_id_tensor else None
    )
    in_names: list[str] = []
    out_names: list[str] = []
    out_avals = []
    for alloc in nc.m.functions[0].allocations:
        if not isinstance(alloc, mybir.MemoryLocationSet):
            continue
        name = alloc.memorylocations[0].name
        if alloc.kind == "ExternalInput":
            if name != partition_name:
                in_names.append(name)
        elif alloc.kind == "ExternalOutput":
            out_names.append(name)
            shape = tuple(alloc.tensor_shape)
            dtype = mybir.dt.np(alloc.dtype)
            out_avals.append(jax.core.ShapedArray(shape, dtype))
    n_params = len(in_names)
    n_outs = len(out_names)
    all_names = list(in_names) + list(out_names)
    if partition_name is not None:
        all_names.append(partition_name)
    all_names = tuple(all_names)

    def _body(*args):
        operands = list(args)
        if partition_name is not None:
            operands.append(bass2jax.partition_id_tensor())
        outs = bass2jax._bass_exec_p.bind(
            *operands,
            out_avals=tuple(out_avals),
            in_names=all_names,
            out_names=tuple(out_names),
            lowering_input_output_aliases=(),
            sim_require_finite=True,
            sim_require_nnan=True,
            nc=nc,
        )
        return tuple(outs)

    devices = jax.devices()[:NCORES]
    mesh = Mesh(np.asarray(devices), ("core",))
    pspec = PartitionSpec("core")
    sharded = jax.jit(
        shard_map(
            _body,
            mesh=mesh,
            in_specs=(pspec,) * (n_params + n_outs),
            out_specs=(pspec,) * n_outs,
            check_rep=False,
        ),
        donate_argnums=tuple(range(n_params, n_params + n_outs)),
        keep_unused=True,
    )
    B = rows * NCORES
    make_donor = jax.jit(
        lambda: jnp.zeros((B, DIM), jnp.float16),
        out_shardings=NamedSharding(mesh, pspec),
    )
    rt = {
        "sharded": sharded,
        "donor": make_donor(),
        "in_names": in_names,
        "sharding": NamedSharding(mesh, pspec),
    }
    _rt_cache[rows] = rt
    return rt


def _cast_parallel(src: np.ndarray, dst: np.ndarray, nthreads: int = 8):
    """dst[:] = src (dtype-converting), parallelized over row slices."""
    from concurrent.futures import ThreadPoolExecutor

    n = src.shape[0]
    step = (n + nthreads - 1) // nthreads
    def work(i):
        np.copyto(dst[i : i + step], src[i : i + step], casting="unsafe")
    with ThreadPoolExecutor(nthreads) as ex:
        list(ex.map(work, range(0, n, step)))
    return dst


_stage16: dict = {}


def kernel(state: np.ndarray, params: np.ndarray, t: np.ndarray = None) -> np.ndarray:
    state = np.asarray(state)
    params = np.asarray(params, dtype=np.float32)
    B = state.shape[0]
    rows = B // NCORES
    rt = _get_runtime(rows)

    if state.dtype == np.float16:
        state16 = np.ascontiguousarray(state)
    else:
        if B not in _stage16:
            _stage16[B] = np.empty((B, DIM), np.float16)
        state16 = _cast_parallel(state, _stage16[B])
    pbs1 = make_pbs(params)
    pbs_all = np.ascontiguousarray(
        np.broadcast_to(pbs1[None], (NCORES, P, 3, PBW))
    ).reshape(NCORES * P, 3, PBW)

    (out16,) = rt["sharded"](state16, pbs_all, rt["donor"])
    rt["donor"] = out16  # donation chain: reuse this device buffer next call
    host16 = np.asarray(out16)
    out32 = np.empty((B, DIM), np.float32)
    return _cast_parallel(host16, out32)


# revision 36
# speedup vs baseline: 1.4276x; 1.3333x over previous
"""Lorenz-96 vector field kernel for Trainium2 (8 NeuronCores, SPMD data-parallel).

field[..., i] = p0[i]*(state[i+1] - state[i-2])*state[i-1] - p1[i]*state[i] + p2[i]
(circular along the last axis, dim=256)

Sharding: batch axis (262144 rows) split evenly across 8 cores; params replicated.

This version runs the whole pipeline in fp16 (inputs are cast on the host):
halving the dtype halves both the HBM traffic on-device (memory-bound regime)
and the host<->device transfer volume, while keeping the relative error around
1e-3 - far below the 2e-2 gate.

Per-core layout: each SBUF partition holds R=32 batch rows as one flat stream of
R*259 halved floats: every row is [halo2 | 256 cols | halo1] where the 3-wide
halo carries the circular wrap (s[254], s[255] on the left, s[0] on the right).
All shifted stencil operands are contiguous flat views of the stream, so every
elementwise op uses the plain 2D tensor_tensor/STT encodings. Halo lanes compute
garbage that is never stored - the output DMA reads only the 256 real columns.

Engine split (6 irreducible two-tensor elementwise ops, balanced by the
cost-model rates DVE fp16 2x-mode 0.52 ns/elem vs GPSIMD 1.98 ns/elem):
  GPSIMD: diff = s[+1]-s[-2], plus ~12% of ops 5,6
  DVE:    um1 = p0*s[-1]; z = diff*um1; v' = (-p1)*s; ~88% of
          f = z + v' and out = f + p2   (p1 negated on host so op5 is an add)
  ACT:    per-tile halo fills + 1/3 of the one-time param-stream build
          (the other 2 planes build on DVE so tile 0 isn't blocked)
The param streams (259-periodic) are built on-device from a tiny [128,3,260]
upload so the wire only carries ~1.6 MB of params instead of ~50 MB.
(Tried and rejected: scalar_tensor_tensor on GPSIMD - walrus rejects
TensorScalarPtr on Pool; SWDGE accumulate-ADD DMAs for ops 5/6 - the
per-DMA DGE+semaphore latency in the dependency chain cost more than the
engine time saved.)

Host runner: a cached jax.jit(shard_map(...)) around the bass custom call - no
per-call retracing, no per-call 256 MB zero upload (the output buffer is
donation-chained from the previous call; the first one is created on-device).
"""

import numpy as np

import concourse.bass as bass
import concourse.mybir as mybir
from concourse.tile import TileContext
from concourse.vector_clock import ScopedClock, VectorClock


class SplitDrainTileContext(TileContext):
    """The kernel-tail Drain aggregates one sem wait per outstanding proc
    (compute engines + every HWDGE queue used); walrus rejects instructions
    with more than a couple of encoded waits. Pre-observe each proc with its
    own single-wait SP nop so the real drain needs none."""

    def _drain_and_barrier(self, tick_clock, wait_clock):
        full = tick_clock.global_clock
        n = len(list(full))
        for p in range(n):
            if full[p] == 0:
                continue
            partial = VectorClock([full[q] if q == p else 0 for q in range(n)])
            nop = self.nc.sync.nop(nofuse=True)
            wait_clock.add_sem_waits(nop.ins, ScopedClock({None: partial}))
        self.nc.sync.drain()
        self.nc.all_engine_barrier()
        assert self.sems is not None
        popped = self.nc._tile_sem_poison_stack.pop()
        assert popped is self._sem_poison
        self.nc.clear_and_free_semaphores(list(self.sems.allocated().values()))
        self.nc.all_engine_barrier()


def _split_waits(nc, limit: int = 1):
    """Post-lowering pass: walrus caps encoded sem waits per instruction
    (TT allows 1, DMACopy ~2). Move excess waits onto same-engine NoOps
    inserted immediately before the instruction - sequencers issue in
    order, so waiting earlier on the same stream preserves ordering."""
    for bb in nc.m.functions[0].blocks:
        il = bb.instructions
        i = 0
        while i < len(il):
            ins = il[i]
            si = getattr(ins, "sync_info", None)
            if si is not None and len(si.on_wait) > limit:
                waits = list(si.on_wait)
                keep, excess = waits[-limit:], waits[:-limit]
                for j, w in enumerate(excess):
                    nop = mybir.InstNoOp(
                        name=f"{ins.name}-wsplit{j}", ins=[], outs=[]
                    )
                    nop.engine = ins.engine
                    nop.sync_info = mybir.SyncInfo(on_wait=[w], on_update=[])
                    il.insert(i, nop)
                    i += 1
                ins.sync_info = mybir.SyncInfo(on_wait=keep, on_update=si.on_update)
            i += 1


P = 128          # SBUF partitions
DIM = 256        # Lorenz-96 dimension (stencil axis, unsharded)
EXT = DIM + 3    # per-row stream width incl. halo
PBW = 260        # padded per-plane width of the uploaded param periods (even)
NCORES = 8
R = 16           # batch rows per partition per tile
F16 = mybir.dt.float16
ADD = mybir.AluOpType.add


DVE_FRAC = 0.876  # DVE share of the op5/op6 lanes (rest on GPSIMD)


def build_nc(rows: int, r: int = R, dve_frac: float = DVE_FRAC,
             split_waits: bool = True, bufs: int = 3, accum_tail: bool = False,
             ext_bufs: int | None = None):
    """Build the per-core Bass program. `rows` = batch rows per core."""
    assert rows % (P * r) == 0
    nt = rows // (P * r)
    W = r * EXT          # flat stream width per partition (real data)
    WP = W + 4           # allocated width (views may peek past W; keep even)
    G0, G1 = 2, W - 1    # compute range (shifts -2..+1 stay in bounds)
    # flat DVE/GPSIMD split point (4-byte aligned for the 16-bit 2x mode)
    SD = G0 + int(round(dve_frac * (G1 - G0) / 2)) * 2

    nc = bass.Bass()
    st = nc.declare_dram_parameter("state", [rows, DIM], F16, isOutput=False)
    pbs = nc.declare_dram_parameter("pbs", [P, 3, PBW], F16, isOutput=False)
    out = nc.declare_dram_parameter("out", [rows, DIM], F16, isOutput=True)

    st_t = st.rearrange("(n p r) d -> n p r d", p=P, r=r)
    out_t = out.rearrange("(n p r) d -> n p r d", p=P, r=r)

    with SplitDrainTileContext(nc) as tc:
        with (
            tc.tile_pool(name="pp", bufs=1) as ppool,
            tc.tile_pool(name="ext", bufs=ext_bufs or bufs) as extpool,
            tc.tile_pool(name="mid", bufs=bufs) as midpool,
            tc.tile_pool(name="op", bufs=bufs) as opool,
        ):
            # --- one-time param-stream build -------------------------------
            # Upload one 259-period per plane (padded to 260), then double it
            # out to the full r-period stream on the (otherwise idle) ACT.
            pbsb = ppool.tile([P, 3 * PBW], F16)
            nc.sync.dma_start(out=pbsb[:], in_=pbs.rearrange("p a w -> p (a w)"))
            pbt = ppool.tile([P, 3 * WP], F16)

            def build_plane(k, eng_copy):
                base = k * WP
                eng_copy(
                    pbt[:, base : base + EXT],
                    pbsb[:, k * PBW : k * PBW + EXT],
                )
                ln = EXT
                while ln < W:
                    cp = min(ln, W - ln)
                    eng_copy(
                        pbt[:, base + ln : base + ln + cp],
                        pbt[:, base : base + cp],
                    )
                    ln += cp

            # planes 0,1 on DVE (needed first, DVE 4x fp16 copies are cheap);
            # plane 2 on ACT, emitted after tile 0's halo fill below so the
            # first tile's stencil ops aren't stuck behind the param build.
            build_plane(0, nc.vector.tensor_copy)
            build_plane(1, nc.vector.tensor_copy)
            P0 = pbt[:, 0 * WP + G0 : 0 * WP + G1]
            P1 = pbt[:, 1 * WP + G0 : 1 * WP + G1]  # holds -p1 (host-negated)
            P2 = pbt[:, 2 * WP + G0 : 2 * WP + G1]

            for i in range(nt):
                ext = extpool.tile([P, WP], F16, tag="ext")
                e3 = ext[:, 0:W].rearrange("p (r c) -> p r c", c=EXT)
                nc.sync.dma_start(out=e3[:, :, 2 : DIM + 2], in_=st_t[i])
                # halo fill on ACT: left 2 cols = state[254:256], right = state[0]
                nc.scalar.copy(e3[:, :, 0:2], e3[:, :, DIM : DIM + 2])
                nc.scalar.copy(e3[:, :, DIM + 2 : DIM + 3], e3[:, :, 2:3])
                if i == 0:
                    build_plane(2, nc.scalar.copy)

                A = ext[:, G0:G1]              # s[c]
                Am1 = ext[:, G0 - 1 : G1 - 1]  # s[c-1]
                Am2 = ext[:, G0 - 2 : G1 - 2]  # s[c-2]
                Ap1 = ext[:, G0 + 1 : G1 + 1]  # s[c+1]

                um1 = midpool.tile([P, WP], F16, tag="um1")
                diff = midpool.tile([P, WP], F16, tag="diff")
                ot = opool.tile([P, WP], F16, tag="o")

                # op1 (GPSIMD): diff[c] = s[c+1] - s[c-2]
                nc.gpsimd.tensor_sub(diff[:, G0:G1], Ap1, Am2)
                # op2 (DVE): um1[c] = p0[c] * s[c-1]
                nc.vector.tensor_mul(um1[:, G0:G1], Am1, P0)
                # op3 (DVE): z = diff * um1   (in-place into um1)
                nc.vector.tensor_mul(um1[:, G0:G1], diff[:, G0:G1], um1[:, G0:G1])
                # op4 (DVE): v'[c] = -p1[c] * s[c]  (reuse diff tile)
                nc.vector.tensor_mul(diff[:, G0:G1], A, P1)
                if accum_tail:
                    # op6' (split at SD): t = z + p2
                    nc.vector.tensor_add(
                        ot[:, G0:SD], um1[:, G0:SD],
                        pbt[:, 2 * WP + G0 : 2 * WP + SD],
                    )
                    nc.gpsimd.tensor_add(
                        ot[:, SD:G1], um1[:, SD:G1],
                        pbt[:, 2 * WP + SD : 2 * WP + G1],
                    )
                    # op5' (SDMA CCE): out = t + v'. Only the out-DMA consumes
                    # this, so no engine ever stalls on the accum latency.
                    nc.gpsimd.dma_start(
                        out=ot[:, G0:G1], in_=diff[:, G0:G1], accum_op=ADD
                    )
                else:
                    # op5 (split at SD): f = z + v'  (in-place into um1)
                    nc.vector.tensor_add(
                        um1[:, G0:SD], um1[:, G0:SD], diff[:, G0:SD]
                    )
                    nc.gpsimd.tensor_add(
                        um1[:, SD:G1], um1[:, SD:G1], diff[:, SD:G1]
                    )
                    # op6 (split at SD): out = f + p2
                    nc.vector.tensor_add(
                        ot[:, G0:SD], um1[:, G0:SD],
                        pbt[:, 2 * WP + G0 : 2 * WP + SD],
                    )
                    nc.gpsimd.tensor_add(
                        ot[:, SD:G1], um1[:, SD:G1],
                        pbt[:, 2 * WP + SD : 2 * WP + G1],
                    )

                o3 = ot[:, 0:W].rearrange("p (r c) -> p r c", c=EXT)
                nc.sync.dma_start(out=out_t[i], in_=o3[:, :, 2 : DIM + 2])

    if split_waits:
        _split_waits(nc)
    return nc


def make_pbs(params: np.ndarray) -> np.ndarray:
    """Host-side param prep: one 259-period per plane, padded to 260,
    broadcast to 128 partitions. Tiny (~200 KB fp16 per core).
    p1 is negated so the on-device '- p1*s' term becomes a tensor add."""
    pp = np.asarray(params, dtype=np.float32).copy()
    pp[1] = -pp[1]
    row = np.zeros((3, PBW), np.float16)
    row[:, 2 : DIM + 2] = pp.astype(np.float16)
    return np.ascontiguousarray(np.broadcast_to(row[None], (P, 3, PBW)))


_rt_cache: dict = {}


def _get_runtime(rows: int):
    """Build-and-cache the jitted SPMD executable for a per-core row count."""
    if rows in _rt_cache:
        return _rt_cache[rows]

    import jax
    import jax.numpy as jnp
    from jax.sharding import Mesh, PartitionSpec, NamedSharding

    try:
        from jax.experimental.shard_map import shard_map
    except ImportError:  # newer jax
        from jax import shard_map  # type: ignore

    from concourse import bass2jax

    nc = build_nc(rows)
    bass2jax.install_neuronx_cc_hook()

    partition_name = (
        nc.partition_id_tensor.name if nc.partition_id_tensor else None
    )
    in_names: list[str] = []
    out_names: list[str] = []
    out_avals = []
    for alloc in nc.m.functions[0].allocations:
        if not isinstance(alloc, mybir.MemoryLocationSet):
            continue
        name = alloc.memorylocations[0].name
        if alloc.kind == "ExternalInput":
            if name != partition_name:
                in_names.append(name)
        elif alloc.kind == "ExternalOutput":
            out_names.append(name)
            shape = tuple(alloc.tensor_shape)
            dtype = mybir.dt.np(alloc.dtype)
            out_avals.append(jax.core.ShapedArray(shape, dtype))
    n_params = len(in_names)
    n_outs = len(out_names)
    all_names = list(in_names) + list(out_names)
    if partition_name is not None:
        all_names.append(partition_name)
    all_names = tuple(all_names)

    def _body(*args):
        operands = list(args)
        if partition_name is not None:
            operands.append(bass2jax.partition_id_tensor())
        outs = bass2jax._bass_exec_p.bind(
            *operands,
            out_avals=tuple(out_avals),
            in_names=all_names,
            out_names=tuple(out_names),
            lowering_input_output_aliases=(),
            sim_require_finite=True,
            sim_require_nnan=True,
            nc=nc,
        )
        return tuple(outs)

    devices = jax.devices()[:NCORES]
    mesh = Mesh(np.asarray(devices), ("core",))
    pspec = PartitionSpec("core")
    sharded = jax.jit(
        shard_map(
            _body,
            mesh=mesh,
            in_specs=(pspec,) * (n_params + n_outs),
            out_specs=(pspec,) * n_outs,
            check_rep=False,
        ),
        donate_argnums=tuple(range(n_params, n_params + n_outs)),
        keep_unused=True,
    )
    B = rows * NCORES
    make_donor = jax.jit(
        lambda: jnp.zeros((B, DIM), jnp.float16),
        out_shardings=NamedSharding(mesh, pspec),
    )
    rt = {
        "sharded": sharded,
        "donor": make_donor(),
        "in_names": in_names,
        "sharding": NamedSharding(mesh, pspec),
    }
    _rt_cache[rows] = rt
    return rt


_cast_pool = None


def _cast_parallel(src: np.ndarray, dst: np.ndarray, nthreads: int = 8):
    """dst[:] = src (dtype-converting), parallelized over row slices."""
    global _cast_pool
    if _cast_pool is None:
        from concurrent.futures import ThreadPoolExecutor

        _cast_pool = ThreadPoolExecutor(nthreads)
    n = src.shape[0]
    step = (n + nthreads - 1) // nthreads
    def work(i):
        np.copyto(dst[i : i + step], src[i : i + step], casting="unsafe")
    list(_cast_pool.map(work, range(0, n, step)))
    return dst


_stage16: dict = {}


def kernel(state: np.ndarray, params: np.ndarray, t: np.ndarray = None) -> np.ndarray:
    state = np.asarray(state)
    params = np.asarray(params, dtype=np.float32)
    B = state.shape[0]
    rows = B // NCORES
    rt = _get_runtime(rows)

    if state.dtype == np.float16:
        state16 = np.ascontiguousarray(state)
    else:
        if B not in _stage16:
            _stage16[B] = np.empty((B, DIM), np.float16)
        state16 = _cast_parallel(state, _stage16[B])
    pbs1 = make_pbs(params)
    pbs_all = np.ascontiguousarray(
        np.broadcast_to(pbs1[None], (NCORES, P, 3, PBW))
    ).reshape(NCORES * P, 3, PBW)

    (out16,) = rt["sharded"](state16, pbs_all, rt["donor"])
    rt["donor"] = out16  # donation chain: reuse this device buffer next call
    host16 = np.asarray(out16)
    out32 = np.empty((B, DIM), np.float32)
    return _cast_parallel(host16, out32)
